# revision 14
# baseline (speedup 1.0000x reference)
"""DepletionLSTM Trainium2 kernel.

Self-contained: builds a Bass/Tile kernel for the 2-layer-LSTM network,
shards the batch over 8 NeuronCores (pure data parallelism), runs via
PJRT/axon, returns the full [8192, 30] float32 output.

Strategy (per core, 1024 batch):
- Everything resident in SBUF; no DRAM round-trips for activations.
- Feature-major layout: activations are [H=128 partitions, batch] tiles.
- Input-projection LayerNorm stats are computed in a prepass directly in
  [T=90 partitions, batch] layout using the quadratic-form identity
  sum_h p_h^2 = x^T (W^T W) x + 2 (W^T b)^T x + |b|^2 (F=7 is tiny, so the
  F-contractions are unrolled on the vector engine).  rsqrt is batched into
  a single Sqrt activation so the ACT table never switches inside the loop.
- Per step: x_t is PE-transposed to feature-major, the projection + bias +
  (-mean) rank-1 terms accumulate in PSUM, rstd is partition-broadcast via
  DMA and applied by one DVE multiply; each LSTM layer is 4 accumulating
  gate matmul pairs (input fold + recurrent), 5 ACT transcendentals with
  the gate bias folded into the activation bias, and 4 DVE elementwise ops.
- Matmul operands use float32r (fp32 bytes, single-pass PE) for speed.

PSUM (8 banks): "pg" gates/head 2x[128,1024] (4), "pp" projection [128,1024]
(2), "pxt" transposes/misc 2x[7..128,<=512] (2).
"""
import sys
sys.path.insert(0, '/opt/trn_rl_repo')

import numpy as np

B, T, F, H, D1, D2, OUT = 8192, 90, 7, 128, 128, 64, 30
NCORES = 8
BL = B // NCORES
G4 = 4 * H
NH = BL // 512
QB = BL // 128
EPS = 1e-5
MMDT = "float32r"


def _build(nc, T_steps=T, mmdt_name=MMDT, dbg=False):
    import concourse.tile as tile
    from concourse import mybir
    from concourse.masks import make_identity

    f32 = mybir.dt.float32
    mmdt = getattr(mybir.dt, mmdt_name)
    AF = mybir.ActivationFunctionType
    ALU = mybir.AluOpType

    # ---------------- DRAM I/O ----------------
    x_d = nc.dram_tensor("x", [BL, T, F], f32, kind="ExternalInput")
    W_in_d = nc.dram_tensor("W_in", [H, F], f32, kind="ExternalInput")
    b_in_d = nc.dram_tensor("b_in", [H], f32, kind="ExternalInput")
    g_in_d = nc.dram_tensor("g_in", [H], f32, kind="ExternalInput")
    be_in_d = nc.dram_tensor("be_in", [H], f32, kind="ExternalInput")
    Wih_d = [nc.dram_tensor("Wih0", [G4, H], f32, kind="ExternalInput"),
             nc.dram_tensor("Wih1", [G4, H], f32, kind="ExternalInput")]
    Whh_d = [nc.dram_tensor("Whh0", [G4, H], f32, kind="ExternalInput"),
             nc.dram_tensor("Whh1", [G4, H], f32, kind="ExternalInput")]
    bih_d = [nc.dram_tensor("bih0", [G4], f32, kind="ExternalInput"),
             nc.dram_tensor("bih1", [G4], f32, kind="ExternalInput")]
    bhh_d = [nc.dram_tensor("bhh0", [G4], f32, kind="ExternalInput"),
             nc.dram_tensor("bhh1", [G4], f32, kind="ExternalInput")]
    g_ln_d = nc.dram_tensor("g_ln", [H], f32, kind="ExternalInput")
    be_ln_d = nc.dram_tensor("be_ln", [H], f32, kind="ExternalInput")
    W_d1_d = nc.dram_tensor("W_d1", [D1, H], f32, kind="ExternalInput")
    b_d1_d = nc.dram_tensor("b_d1", [D1], f32, kind="ExternalInput")
    W_d2_d = nc.dram_tensor("W_d2", [D2, D1], f32, kind="ExternalInput")
    b_d2_d = nc.dram_tensor("b_d2", [D2], f32, kind="ExternalInput")
    W_d3_d = nc.dram_tensor("W_d3", [OUT, D2], f32, kind="ExternalInput")
    b_d3_d = nc.dram_tensor("b_d3", [OUT], f32, kind="ExternalInput")
    out_d = nc.dram_tensor("out", [BL, OUT], f32, kind="ExternalOutput")
    if dbg:
        dbg_xfm = nc.dram_tensor("dbg_xfm", [F, BL], f32, kind="ExternalOutput")
        dbg_stats = nc.dram_tensor("dbg_stats", [2, BL], f32, kind="ExternalOutput")
        dbg_x0 = nc.dram_tensor("dbg_x0", [H, BL], f32, kind="ExternalOutput")
        dbg_h0 = nc.dram_tensor("dbg_h0", [H, BL], f32, kind="ExternalOutput")
        dbg_c0 = nc.dram_tensor("dbg_c0", [H, BL], f32, kind="ExternalOutput")
        dbg_pp = nc.dram_tensor("dbg_pp", [H, BL], f32, kind="ExternalOutput")
        dbg_rbc = nc.dram_tensor("dbg_rbc", [2, BL], f32, kind="ExternalOutput")

    import contextlib
    with tile.TileContext(nc) as tc, contextlib.ExitStack() as ctx:
        singles = ctx.enter_context(tc.tile_pool(name="singles", bufs=1))
        trans = ctx.enter_context(tc.tile_pool(name="trans", bufs=2))
        small = ctx.enter_context(tc.tile_pool(name="small", bufs=2))
        ps_pg = ctx.enter_context(tc.tile_pool(name="ps_pg", bufs=2, space="PSUM"))
        ps_pp = ctx.enter_context(tc.tile_pool(name="ps_pp", bufs=1, space="PSUM"))
        ps_px = ctx.enter_context(tc.tile_pool(name="ps_px", bufs=2, space="PSUM"))
        dpool = ctx.enter_context(tc.tile_pool(name="dpool", bufs=1, space="DRAM"))

        def pg_tile(shape, name):
            return ps_pg.tile(shape, f32, tag="pg", name=name)

        def pp_tile(shape, name):
            return ps_pp.tile(shape, f32, tag="pp", name=name)

        def px_tile(shape, name):
            return ps_px.tile(shape, f32, tag="pxt", name=name)

        def R(ap):
            return ap

        # ---------------- constants ----------------
        ident = singles.tile([128, 128], f32)
        make_identity(nc, ident)
        ones_row = singles.tile([1, 512], f32)
        nc.vector.memset(ones_row, 1.0)
        ones_col = singles.tile([128, 1], f32)
        nc.vector.memset(ones_col, 1.0)
        eps_col = singles.tile([T, 1], f32)
        nc.vector.memset(eps_col, EPS)

        def load_col(dram_vec, n, name):
            t_ = singles.tile([n, 1], f32, name=name, tag=name)
            nc.sync.dma_start(out=t_, in_=dram_vec[:].rearrange("(p o) -> p o", o=1))
            return t_

        g_in_c = load_col(g_in_d, H, "g_in_c")
        be_in_c = load_col(be_in_d, H, "be_in_c")
        b_in_c = load_col(b_in_d, H, "b_in_c")
        g_ln_c = load_col(g_ln_d, H, "g_ln_c")
        be_ln_c = load_col(be_ln_d, H, "be_ln_c")
        b_d1_c = load_col(b_d1_d, D1, "b_d1_c")
        b_d2_c = load_col(b_d2_d, D2, "b_d2_c")
        b_d3_c = load_col(b_d3_d, OUT, "b_d3_c")
        b_in_row = singles.tile([1, H], f32)
        nc.sync.dma_start(out=b_in_row, in_=b_in_d[:].rearrange("(o p) -> o p", o=1))

        # ---------------- weights: load + PE-transpose ----------------
        def transpose_to(dst, src_ap, p, fdim):
            pt = pp_tile([fdim, p], "tr_ps")
            nc.tensor.transpose(pt, src_ap, ident[:p, :p])
            nc.vector.tensor_copy(out=dst, in_=pt)

        w_in_raw = singles.tile([H, F], f32)
        nc.sync.dma_start(out=w_in_raw, in_=W_in_d[:, :])
        w_inT = singles.tile([F, H], mmdt)
        transpose_to(w_inT, w_in_raw, H, F)

        wihT0f = singles.tile([H, 4, H], f32)
        wihT, whhT = [], []
        for L in range(2):
            wt = singles.tile([H, 4, H], mmdt, name=f"wihT{L}", tag=f"wihT{L}")
            ht = singles.tile([H, 4, H], mmdt, name=f"whhT{L}", tag=f"whhT{L}")
            for cc in range(4):
                raw = trans.tile([H, H], f32, tag="u", name="raw")
                nc.sync.dma_start(out=raw, in_=Wih_d[L][cc * H:(cc + 1) * H, :])
                pt_w = pp_tile([H, H], "tr_ps_w")
                nc.tensor.transpose(pt_w, raw, ident)
                nc.vector.tensor_copy(out=wt[:, cc, :], in_=pt_w)
                if L == 0:
                    nc.vector.tensor_copy(out=wihT0f[:, cc, :], in_=pt_w)
                raw2 = trans.tile([H, H], f32, tag="v_", name="raw2")
                nc.sync.dma_start(out=raw2, in_=Whh_d[L][cc * H:(cc + 1) * H, :])
                transpose_to(ht[:, cc, :], raw2, H, H)
            wihT.append(wt)
            whhT.append(ht)

        # gate biases beff[L] [128, 4]; layer-0 gains Wih0 @ be_in (beta fold)
        beff = []
        for L in range(2):
            bt_ = singles.tile([H, 4], f32, name=f"beff{L}", tag=f"beff{L}")
            bih_sb = small.tile([H, 4], f32, tag="bload", name="bih_sb")
            nc.sync.dma_start(out=bih_sb,
                              in_=bih_d[L][:].rearrange("(c p) -> p c", p=H))
            bhh_sb = small.tile([H, 4], f32, tag="bload2", name="bhh_sb")
            nc.sync.dma_start(out=bhh_sb,
                              in_=bhh_d[L][:].rearrange("(c p) -> p c", p=H))
            nc.vector.tensor_add(out=bt_, in0=bih_sb, in1=bhh_sb)
            beff.append(bt_)
        for cc in range(4):
            pb = px_tile([H, 1], "pb")
            nc.tensor.matmul(pb, wihT0f[:, cc, :], be_in_c, start=True, stop=True)
            nc.vector.tensor_add(out=beff[0][:, cc:cc + 1],
                                 in0=beff[0][:, cc:cc + 1], in1=pb)
        # gamma-fold layer-0 input weights (rows scaled by g_in)
        nc.vector.tensor_scalar_mul(
            out=wihT[0][:, :, :].rearrange("p c m -> p (c m)"),
            in0=wihT[0][:, :, :].rearrange("p c m -> p (c m)"),
            scalar1=g_in_c)

        wd1T = singles.tile([H, D1], f32)
        wd1_raw = trans.tile([D1, H], f32, tag="u", name="wd1_raw")
        nc.sync.dma_start(out=wd1_raw, in_=W_d1_d[:, :])
        transpose_to(wd1T, wd1_raw, D1, H)
        wd2T = singles.tile([D1, D2], f32)
        wd2_raw = trans.tile([D2, D1], f32, tag="v_", name="wd2_raw")
        nc.sync.dma_start(out=wd2_raw, in_=W_d2_d[:, :])
        transpose_to(wd2T, wd2_raw, D2, D1)
        wd3T = singles.tile([D2, OUT], f32)
        wd3_raw = trans.tile([OUT, D2], f32, tag="u", name="wd3_raw")
        nc.sync.dma_start(out=wd3_raw, in_=W_d3_d[:, :])
        transpose_to(wd3T, wd3_raw, OUT, D2)

        # ---------------- x loads ----------------
        # loop layout: xrow[p, t, q, f] = x[128q+p, t, f]
        xrow_all = singles.tile([128, T, QB, F], f32)
        nc.sync.dma_start(
            out=xrow_all,
            in_=x_d[:, :, :].rearrange("(q p) t f -> p t q f", p=128))
        # prepass layout: x_tm[t, q, p, f] = x[128q+p, t, f]
        x_tm = singles.tile([T, QB, 128, F], f32)
        nc.sync.dma_start(
            out=x_tm,
            in_=x_d[:, :, :].rearrange("(q p) t f -> t q p f", p=128))

        # ---------------- prepass: LN stats in [T, BL] layout ----------------
        # p' = W_in x + b_in per (h | b,t); over h:
        #   sum p'   = wsum . x + bsum
        #   sum p'^2 = x^T M x + 2 l^T x + c0,  M = W^T W, l = W^T b, c0=|b|^2
        p_m = pp_tile([F, F], "stat_m")
        nc.tensor.matmul(p_m, w_in_raw, w_in_raw, start=True, stop=True)
        p_ws = px_tile([1, F], "stat_ws")
        nc.tensor.matmul(p_ws, ones_col, w_in_raw, start=True, stop=True)
        p_l = px_tile([1, F], "stat_l")
        nc.tensor.matmul(p_l, b_in_c, w_in_raw, start=True, stop=True)
        p_sc = px_tile([1, 2], "stat_sc")
        nc.tensor.matmul(p_sc[:, 0:1], b_in_c, b_in_c, start=True, stop=False,
                         skip_group_check=True)
        nc.tensor.matmul(p_sc[:, 1:2], ones_col, b_in_c, start=False, stop=True,
                         skip_group_check=True)
        m_sb = small.tile([F, F], f32, tag="m_sb", name="m_sb")
        nc.vector.tensor_copy(out=m_sb, in_=p_m)
        ws_sb = small.tile([1, F], f32, tag="ws_sb", name="ws_sb")
        nc.vector.tensor_copy(out=ws_sb, in_=p_ws)
        l_sb = small.tile([1, F], f32, tag="l_sb", name="l_sb")
        nc.vector.tensor_copy(out=l_sb, in_=p_l)
        sc_sb = small.tile([1, 2], f32, tag="sc_sb", name="sc_sb")
        nc.vector.tensor_copy(out=sc_sb, in_=p_sc)
        # stage stat constants to DRAM, then partition-broadcast them back
        stat_dram = dpool.tile([F + 2, F * F], f32)
        nc.sync.dma_start(out=stat_dram[0:1, :].rearrange("o (a b) -> (o a) b", a=F),
                          in_=m_sb)
        nc.sync.dma_start(out=stat_dram[F:F + 1, 0:F], in_=ws_sb)
        nc.sync.dma_start(out=stat_dram[F:F + 1, F:2 * F], in_=l_sb)
        nc.sync.dma_start(out=stat_dram[F + 1:F + 2, 0:2], in_=sc_sb)
        wbc = singles.tile([T, F], f32)
        nc.gpsimd.dma_start(out=wbc, in_=stat_dram[F:F + 1, 0:F].to_broadcast([T, F]))
        lbc = singles.tile([T, F], f32)
        nc.gpsimd.dma_start(out=lbc,
                            in_=stat_dram[F:F + 1, F:2 * F].to_broadcast([T, F]))
        mbc = singles.tile([T, F * F], f32)
        nc.gpsimd.dma_start(out=mbc, in_=stat_dram[0:1, :].to_broadcast([T, F * F]))
        scbc = singles.tile([T, 2], f32)
        nc.gpsimd.dma_start(out=scbc,
                            in_=stat_dram[F + 1:F + 2, 0:2].to_broadcast([T, 2]))

        def xf(fi):
            return x_tm[:T_steps, :, :, fi].rearrange("t q p -> t (q p)")

        TS = T_steps
        nmu_all = singles.tile([T, BL], f32)
        r_all = singles.tile([T, BL], f32)
        acc = trans.tile([T, BL], f32, tag="sig_i", name="st_acc")
        nc.vector.tensor_scalar_mul(out=acc[:TS], in0=xf(0), scalar1=wbc[:TS, 0:1])
        for fi in range(1, F):
            nc.vector.scalar_tensor_tensor(
                out=acc[:TS], in0=xf(fi), scalar=wbc[:TS, fi:fi + 1],
                in1=acc[:TS], op0=ALU.mult, op1=ALU.add)
        # nmu = -(acc + bsum)/H
        nc.vector.tensor_scalar(out=nmu_all[:TS], in0=acc[:TS],
                                scalar1=scbc[:TS, 1:2], scalar2=-1.0 / H,
                                op0=ALU.add, op1=ALU.mult)
        # quadratic form
        qacc = trans.tile([T, BL], f32, tag="sig_f", name="st_qacc")
        yf = trans.tile([T, BL], f32, tag="tg", name="st_yf")
        tmp = trans.tile([T, BL], f32, tag="sig_o", name="st_tmp")
        for fi in range(F):
            nc.vector.tensor_scalar_mul(out=yf[:TS], in0=xf(0),
                                        scalar1=mbc[:TS, fi * F:fi * F + 1])
            for fj in range(1, F):
                nc.vector.scalar_tensor_tensor(
                    out=yf[:TS], in0=xf(fj),
                    scalar=mbc[:TS, fi * F + fj:fi * F + fj + 1],
                    in1=yf[:TS], op0=ALU.mult, op1=ALU.add)
            nc.vector.tensor_tensor(out=tmp[:TS], in0=xf(fi), in1=yf[:TS],
                                    op=ALU.mult)
            if fi == 0:
                nc.vector.tensor_copy(out=qacc[:TS], in_=tmp[:TS])
            else:
                nc.vector.tensor_add(out=qacc[:TS], in0=qacc[:TS], in1=tmp[:TS])
        # + 2 l.x
        lin = trans.tile([T, BL], f32, tag="u", name="st_lin")
        nc.vector.tensor_scalar_mul(out=lin[:TS], in0=xf(0), scalar1=lbc[:TS, 0:1])
        for fi in range(1, F):
            nc.vector.scalar_tensor_tensor(
                out=lin[:TS], in0=xf(fi), scalar=lbc[:TS, fi:fi + 1],
                in1=lin[:TS], op0=ALU.mult, op1=ALU.add)
        nc.vector.scalar_tensor_tensor(out=qacc[:TS], in0=lin[:TS], scalar=2.0,
                                       in1=qacc[:TS], op0=ALU.mult, op1=ALU.add)
        # var = (q + c0)/H - mu^2 ; r = 1/sqrt(var+eps)
        nc.vector.tensor_scalar(out=qacc[:TS], in0=qacc[:TS],
                                scalar1=scbc[:TS, 0:1], scalar2=1.0 / H,
                                op0=ALU.add, op1=ALU.mult)
        nc.vector.tensor_tensor(out=tmp[:TS], in0=nmu_all[:TS], in1=nmu_all[:TS],
                                op=ALU.mult)
        nc.vector.tensor_sub(out=qacc[:TS], in0=qacc[:TS], in1=tmp[:TS])
        nc.scalar.activation(out=r_all[:TS], in_=qacc[:TS], func=AF.Sqrt,
                             bias=eps_col[:TS], scale=1.0)
        nc.vector.reciprocal(out=r_all[:TS], in_=r_all[:TS])
        nmr_all = singles.tile([T, BL], f32)
        nc.vector.tensor_tensor(out=nmr_all[:TS], in0=nmu_all[:TS],
                                in1=r_all[:TS], op=ALU.mult)
        r_dram = dpool.tile([T, BL], f32)
        nc.sync.dma_start(out=r_dram[:TS], in_=r_all[:TS])
        nmr_dram = dpool.tile([T, BL], f32)
        nc.sync.dma_start(out=nmr_dram[:TS], in_=nmr_all[:TS])

        # ---------------- states ----------------
        h = [singles.tile([H, BL], mmdt, name="h0", tag="h0"),
             singles.tile([H, BL], mmdt, name="h1", tag="h1")]
        c = [singles.tile([H, BL], f32, name="c0", tag="c0"),
             singles.tile([H, BL], f32, name="c1", tag="c1")]
        zinit = trans.tile([H, BL], f32, tag="rbc", name="zinit")
        nc.vector.memset(zinit, 0.0)
        for L in range(2):
            nc.vector.tensor_copy(out=h[L], in_=zinit)
            nc.vector.memset(c[L], 0.0)

        # ---------------- main loop ----------------
        def lstm_step(L, inp, hh_first):
            sig_i = trans.tile([H, BL], f32, tag="sig_i", name="sig_i")
            sig_f = trans.tile([H, BL], f32, tag="sig_f", name="sig_f")
            tg = trans.tile([H, BL], f32, tag="tg", name="tg")
            sig_o = trans.tile([H, BL], f32, tag="sig_o", name="sig_o")
            outs = [sig_i, sig_f, tg, sig_o]
            funcs = [AF.Sigmoid, AF.Sigmoid, AF.Tanh, AF.Sigmoid]
            for gc in range(4):
                pg = pg_tile([H, BL], "pg_gates")
                for hc in range(NH):
                    sl = slice(hc * 512, (hc + 1) * 512)
                    ops = [(wihT[L][:, gc, :], inp), (whhT[L][:, gc, :], h[L])]
                    if hh_first:
                        ops.reverse()
                    nc.tensor.matmul(pg[:, sl], R(ops[0][0]), R(ops[0][1][:, sl]),
                                     start=True, stop=False)
                    nc.tensor.matmul(pg[:, sl], R(ops[1][0]), R(ops[1][1][:, sl]),
                                     start=False, stop=True)
                nc.scalar.activation(out=outs[gc], in_=pg, func=funcs[gc],
                                     bias=beff[L][:, gc:gc + 1], scale=1.0)
            u = trans.tile([H, BL], f32, tag="u", name="u")
            nc.vector.tensor_tensor(out=u, in0=sig_i, in1=tg, op=ALU.mult)
            v_ = trans.tile([H, BL], f32, tag="v_", name="v_")
            nc.vector.tensor_tensor(out=v_, in0=sig_f, in1=c[L], op=ALU.mult)
            nc.vector.tensor_add(out=c[L], in0=u, in1=v_)
            tc_ = trans.tile([H, BL], f32, tag="tc_", name="tc_")
            nc.scalar.activation(out=tc_, in_=c[L], func=AF.Tanh, scale=1.0)
            nc.vector.tensor_tensor(out=h[L], in0=sig_o, in1=tc_, op=ALU.mult)

        for t in range(T_steps):
            # x_t -> feature-major [7, BL] (8 PE transposes, 2 PSUM halves)
            pxs = []
            for half in range(2):
                px = px_tile([F, 512], f"pxt{half}")
                for qi in range(4):
                    q = half * 4 + qi
                    nc.tensor.transpose(
                        px[:, qi * 128:(qi + 1) * 128],
                        xrow_all[:, t, q, :], ident)
                pxs.append(px)
            x_fm = trans.tile([F, BL], mmdt, tag="x_fm", name="x_fm")
            nc.vector.tensor_copy(out=x_fm[:, 0:512], in_=pxs[0])
            nc.vector.tensor_copy(out=x_fm[:, 512:1024], in_=pxs[1])
            # rstd / (-mean*rstd) broadcasts via DMA (partition-stride 0)
            rbc = trans.tile([H, BL], f32, tag="rbc", name="rbc")
            nc.gpsimd.dma_start(out=rbc,
                                in_=r_dram[t:t + 1, :].to_broadcast([H, BL]))
            nmrbc = trans.tile([H, BL], f32, tag="nmrbc", name="nmrbc")
            nc.gpsimd.dma_start(out=nmrbc,
                                in_=nmr_dram[t:t + 1, :].to_broadcast([H, BL]))
            # projection + b_in rank-1 term
            pp = pp_tile([H, BL], "pp_proj")
            for q in range(QB):
                nc.tensor.matmul(
                    pp[:, q * 128:(q + 1) * 128], R(w_inT),
                    R(x_fm[:, q * 128:(q + 1) * 128]),
                    start=(q % 4 == 0), stop=False, skip_group_check=True)
            for hc in range(NH):
                sl = slice(hc * 512, (hc + 1) * 512)
                nc.tensor.matmul(pp[:, sl], b_in_row, ones_row,
                                 start=False, stop=(hc == NH - 1),
                                 skip_group_check=True)
            x0 = trans.tile([H, BL], mmdt, tag="x0", name="x0")
            nc.vector.tensor_tensor(out=x0, in0=pp, in1=rbc, op=ALU.mult)
            nc.vector.tensor_add(out=x0, in0=x0.bitcast(f32), in1=nmrbc)
            lstm_step(0, x0, hh_first=False)
            lstm_step(1, h[0], hh_first=True)
            if dbg and t == 0:
                ppc = trans.tile([H, BL], f32, tag="tc_", name="ppc_dbg")
                nc.vector.tensor_copy(out=ppc, in_=pp)
                nc.sync.dma_start(out=dbg_pp[:, :], in_=ppc)
                nc.sync.dma_start(out=dbg_rbc[0:1, :], in_=rbc[0:1, :])
                nc.sync.dma_start(out=dbg_rbc[1:2, :], in_=nmrbc[13:14, :])
                nc.sync.dma_start(out=dbg_xfm[:, :], in_=x_fm.bitcast(f32))
                nc.sync.dma_start(out=dbg_stats[0:1, :], in_=nmu_all[0:1, :])
                nc.sync.dma_start(out=dbg_stats[1:2, :], in_=r_all[0:1, :])
                nc.sync.dma_start(out=dbg_x0[:, :], in_=x0.bitcast(f32))
                nc.sync.dma_start(out=dbg_h0[:, :], in_=h[0].bitcast(f32))
                nc.sync.dma_start(out=dbg_c0[:, :], in_=c[0])

        # ---------------- head ----------------
        h1f = trans.tile([H, BL], f32, tag="x0", name="h1f")
        nc.vector.tensor_copy(out=h1f, in_=h[1].bitcast(f32))
        sqh = trans.tile([H, BL], f32, tag="sig_f", name="sqh")
        nc.vector.tensor_tensor(out=sqh, in0=h1f, in1=h1f, op=ALU.mult)
        ps_s1 = pp_tile([1, BL], "ps_s1")
        ps_s2 = pp_tile([1, BL], "ps_s2")
        for hc in range(NH):
            sl = slice(hc * 512, (hc + 1) * 512)
            nc.tensor.matmul(ps_s1[:, sl], ones_col, h1f[:, sl],
                             start=True, stop=True, skip_group_check=True)
            nc.tensor.matmul(ps_s2[:, sl], ones_col, sqh[:, sl],
                             start=True, stop=True, skip_group_check=True)
        nmu_h = small.tile([1, BL], f32, tag="nmu_h", name="nmu_h")
        nc.vector.tensor_scalar_mul(out=nmu_h, in0=ps_s1, scalar1=-1.0 / H)
        musq_h = small.tile([1, BL], f32, tag="musq", name="musq_h")
        nc.vector.tensor_tensor(out=musq_h, in0=nmu_h, in1=nmu_h, op=ALU.mult)
        v_h = small.tile([1, BL], f32, tag="v_h", name="v_h")
        nc.vector.tensor_scalar_mul(out=v_h, in0=ps_s2, scalar1=1.0 / H)
        nc.vector.tensor_sub(out=v_h, in0=v_h, in1=musq_h)
        nc.scalar.activation(out=v_h, in_=v_h, func=AF.Sqrt,
                             bias=eps_col[0:1], scale=1.0)
        nc.vector.reciprocal(out=v_h, in_=v_h)
        hstat_dram = dpool.tile([2, BL], f32)
        nc.sync.dma_start(out=hstat_dram[0:1, :], in_=nmu_h)
        nc.sync.dma_start(out=hstat_dram[1:2, :], in_=v_h)
        nmbc = trans.tile([H, BL], f32, tag="rbc", name="nmbc")
        nc.gpsimd.dma_start(out=nmbc, in_=hstat_dram[0:1, :].to_broadcast([H, BL]))
        rhbc = trans.tile([H, BL], f32, tag="sig_i", name="rhbc")
        nc.gpsimd.dma_start(out=rhbc, in_=hstat_dram[1:2, :].to_broadcast([H, BL]))
        t1 = trans.tile([H, BL], f32, tag="tg", name="t1")
        nc.vector.tensor_tensor(out=t1, in0=h1f, in1=nmbc, op=ALU.add)
        t2 = trans.tile([H, BL], f32, tag="sig_o", name="t2")
        nc.vector.tensor_tensor(out=t2, in0=t1, in1=rhbc, op=ALU.mult)
        last = trans.tile([H, BL], f32, tag="u", name="last")
        nc.vector.tensor_scalar(out=last, in0=t2, scalar1=g_ln_c,
                                scalar2=be_ln_c, op0=ALU.mult, op1=ALU.add)
        pd1 = pg_tile([D1, BL], "pd1")
        for hc in range(NH):
            sl = slice(hc * 512, (hc + 1) * 512)
            nc.tensor.matmul(pd1[:, sl], wd1T, last[:, sl], start=True, stop=True,
                             skip_group_check=True)
        d1 = trans.tile([D1, BL], f32, tag="v_", name="d1")
        nc.scalar.activation(out=d1, in_=pd1, func=AF.Relu, bias=b_d1_c, scale=1.0)
        pd2 = pg_tile([D2, BL], "pd2")
        for hc in range(NH):
            sl = slice(hc * 512, (hc + 1) * 512)
            nc.tensor.matmul(pd2[:, sl], wd2T, d1[:, sl], start=True, stop=True,
                             skip_group_check=True)
        d2 = trans.tile([D2, BL], f32, tag="tc_", name="d2")
        nc.scalar.activation(out=d2, in_=pd2, func=AF.Relu, bias=b_d2_c, scale=1.0)
        pd3 = pg_tile([OUT, BL], "pd3")
        for hc in range(NH):
            sl = slice(hc * 512, (hc + 1) * 512)
            nc.tensor.matmul(pd3[:, sl], wd3T, d2[:, sl], start=True, stop=True,
                             skip_group_check=True)
        o3 = trans.tile([OUT, BL], f32, tag="sig_f", name="o3")
        nc.scalar.activation(out=o3, in_=pd3, func=AF.Identity, bias=b_d3_c,
                             scale=1.0)
        outT = singles.tile([128, QB, OUT], f32)
        for q in range(QB):
            pot = px_tile([128, OUT], "pot")
            nc.tensor.transpose(pot, o3[:, q * 128:(q + 1) * 128],
                                ident[:OUT, :OUT])
            nc.vector.tensor_copy(out=outT[:, q, :], in_=pot)
        nc.sync.dma_start(
            out=out_d[:, :].rearrange("(q p) c -> p q c", p=128),
            in_=outT)
    return nc


_CACHE = {}


def _get_runner():
    if "runner" in _CACHE:
        return _CACHE["runner"]
    import jax
    from jax.sharding import Mesh, PartitionSpec
    from jax.experimental.shard_map import shard_map
    import concourse.bacc as bacc
    import concourse.mybir as mybir
    from concourse.bass2jax import install_neuronx_cc_hook, _bass_exec_p, \
        partition_id_tensor

    nc = bacc.Bacc()
    _build(nc)
    nc.compile()
    install_neuronx_cc_hook()

    partition_name = nc.partition_id_tensor.name if nc.partition_id_tensor else None
    in_names, out_names, out_avals, zero_outs = [], [], [], []
    for alloc in nc.m.functions[0].allocations:
        if not isinstance(alloc, mybir.MemoryLocationSet):
            continue
        name = alloc.memorylocations[0].name
        if alloc.kind == "ExternalInput":
            if name != partition_name:
                in_names.append(name)
        elif alloc.kind == "ExternalOutput":
            out_names.append(name)
            shape = tuple(alloc.tensor_shape)
            dtype = mybir.dt.np(alloc.dtype)
            out_avals.append(jax.core.ShapedArray(shape, dtype))
            zero_outs.append(np.zeros(shape, dtype))
    n_params = len(in_names)
    all_in_names = in_names + out_names + ([partition_name] if partition_name else [])

    def _body(*args):
        operands = list(args)
        if partition_name is not None:
            operands.append(partition_id_tensor())
        outs = _bass_exec_p.bind(
            *operands,
            out_avals=tuple(out_avals),
            in_names=tuple(all_in_names),
            out_names=tuple(out_names),
            lowering_input_output_aliases=(),
            sim_require_finite=False,
            sim_require_nnan=False,
            nc=nc,
        )
        return tuple(outs)

    devices = jax.devices()[:NCORES]
    mesh = Mesh(np.asarray(devices), ("core",))
    in_specs = (PartitionSpec("core"),) * (n_params + len(out_names))
    out_specs = (PartitionSpec("core"),) * len(out_names)
    sharded = jax.jit(
        shard_map(_body, mesh=mesh, in_specs=in_specs, out_specs=out_specs,
                  check_rep=False),
        keep_unused=True)
    _CACHE["runner"] = (sharded, in_names, out_names, zero_outs)
    return _CACHE["runner"]


def kernel(**inputs) -> np.ndarray:
    sharded, in_names, out_names, zero_outs = _get_runner()
    inp = {k: np.ascontiguousarray(np.asarray(v), dtype=np.float32)
           for k, v in inputs.items()}

    def core_val(name, ci):
        if name == "x":
            return inp["x"][ci * BL:(ci + 1) * BL]
        return inp[name]

    concat_in = [
        np.concatenate([core_val(n, ci) for ci in range(NCORES)], axis=0)
        for n in in_names
    ]
    concat_zeros = [
        np.zeros((NCORES * z.shape[0], *z.shape[1:]), z.dtype) for z in zero_outs
    ]
    import jax
    out_arrs = sharded(*concat_in, *concat_zeros)
    jax.block_until_ready(out_arrs)
    oi = out_names.index("out")
    full = np.asarray(out_arrs[oi]).reshape(B, OUT)
    return full.astype(np.float32)
